# revision 42
# baseline (speedup 1.0000x reference)
"""Falcon-style MQA attention (71 heads, 1 KV head, RoPE, causal) on 8 TRN2 NeuronCores.

Sharding: tensor-parallel over query heads (9 per core, core 7 has 8 + a zero-pad
head), the single KV head replicated. Per core: QKV projection for its heads
(+KV), RoPE, causal flash-style attention in transposed layout, then a PARTIAL
dense projection over the core's own head rows for all 4544 output columns.
The host sums the 8 bf16 partial outputs in f32 (contraction-sharded dense =
host reduce); no device collective is needed.

v2 layout: everything bf16 on the PE (fp32r attention matmuls were 3x slower
under HAM throttling), softmax reciprocal via the fast custom-DVE op, dense
partials cast to bf16, all DMA issue on the sync queue, and QKV(b1)/dense
emission finely interleaved into the attention instruction streams so the
tensor engine never idles (keeps the HAM clock gate at 2.4 GHz).

Self-contained: hardcodes all shapes; needs only numpy + ml_dtypes + concourse.
"""

import math
from contextlib import ExitStack

import numpy as np
import ml_dtypes

import concourse.bass as bass
import concourse.mybir as mybir
import concourse.tile as tile
from concourse import bacc
from concourse.bass_utils import run_bass_kernel_spmd

NCORES = 8
N, L, D = 2, 1024, 4544
H, DKV = 71, 64
M = N * L                    # 2048 tokens
DP = 4608                    # D padded to 36*128
KT = DP // 128               # 36 contraction tiles for QKV
HPC = 9                      # head slots per core (core 7: 8 real + 1 zero-pad)
QROWS = HPC * DKV            # 576 attention rows per core
QPAD = 640                   # padded to 5*128 for the dense contraction
RROWS = QROWS + 2 * DKV      # 704 fused rows per core (q + k + v)
RC = 6                       # row-chunks of fusedT (5x128 + 64)
MCH = 256                    # QKV token-chunk width
KTG = 6                      # kt-subgroup size (DMA + matmul granularity)
ROPE_BASE = 10000.0

F32 = mybir.dt.float32
F16 = mybir.dt.float16
BF16 = mybir.dt.bfloat16

CCH = [512] * 8 + [448]      # dense column chunks (sum = 4544)


def _build():
    nc = bacc.Bacc("TRN2", target_bir_lowering=False, debug=False, num_devices=NCORES)

    hs_bf = nc.dram_tensor("hs_bf", [DP, M], BF16, kind="ExternalInput")      # hs.T
    wq_bf = nc.dram_tensor("wq_bf", [DP, RROWS], BF16, kind="ExternalInput")  # wq_loc.T
    wd_bf = nc.dram_tensor("wd_bf", [QPAD, D], BF16, kind="ExternalInput")    # wd rows for local heads
    cos2 = nc.dram_tensor("cos2", [128, L], BF16, kind="ExternalInput")
    sin2 = nc.dram_tensor("sin2", [128, L], BF16, kind="ExternalInput")
    tri_in = nc.dram_tensor("tri", [128, 128], BF16, kind="ExternalInput")
    prope2 = nc.dram_tensor("prope2", [128, 128], BF16, kind="ExternalInput")
    ident64 = nc.dram_tensor("ident64", [64, 64], BF16, kind="ExternalInput")
    colones = nc.dram_tensor("colones", [128, 16], BF16, kind="ExternalInput")
    ones1 = nc.dram_tensor("ones1", [1, 64], F16, kind="ExternalInput")
    out = nc.dram_tensor("out", [M, D], BF16, kind="ExternalOutput")
    dbg_rec = nc.dram_tensor("dbg_rec", [1, 512], F32, kind="ExternalOutput")
    dbg_den = nc.dram_tensor("dbg_den", [1, 512], F32, kind="ExternalOutput")
    dbg_fus = nc.dram_tensor("dbg_fus", [128, 512], BF16, kind="ExternalOutput")

    with tile.TileContext(nc) as tc, ExitStack() as top:
        constp = top.enter_context(tc.tile_pool(name="const", bufs=1))
        workp = top.enter_context(tc.tile_pool(name="work", bufs=2))
        hstp = top.enter_context(tc.tile_pool(name="hst", bufs=2))
        otp = top.enter_context(tc.tile_pool(name="ot", bufs=4))
        recp = top.enter_context(tc.tile_pool(name="rec", bufs=2))
        expp = top.enter_context(tc.tile_pool(name="exps", bufs=5))
        bigp = top.enter_context(tc.tile_pool(name="big", bufs=2, space="PSUM"))
        spp = top.enter_context(tc.tile_pool(name="spp", bufs=3, space="PSUM"))
        avp = top.enter_context(tc.tile_pool(name="avp", bufs=3, space="PSUM"))

        # ---- persistent tiles ----
        fusedp = top.enter_context(tc.tile_pool(name="fused", bufs=1))
        fusedT = fusedp.tile([128, RC, M], BF16)
        attnp = top.enter_context(tc.tile_pool(name="attn", bufs=1))
        attn_sb = attnp.tile([128, QPAD // 128, M], BF16)
        wqp = top.enter_context(tc.tile_pool(name="wq", bufs=1))
        wqT = wqp.tile([128, KT, RROWS], BF16)
        wdp = top.enter_context(tc.tile_pool(name="wd", bufs=1))
        wdT2 = wdp.tile([128, QPAD // 128, D], BF16)
        kvp = top.enter_context(tc.tile_pool(name="kv", bufs=1))
        kT_dup = kvp.tile([128, N, L], BF16)
        v_nat = kvp.tile([128, N * 8, DKV + 1], BF16)

        # ---- constants ----
        cosT = constp.tile([128, L], BF16)
        sinT = constp.tile([128, L], BF16)
        tri = constp.tile([128, 128], BF16)
        prope = constp.tile([128, 128], BF16)
        id64 = constp.tile([64, 64], BF16)
        ones_1x64 = constp.tile([1, 64], F16)
        def load_consts():
            # emitted after the first wq groups so the startup-critical
            # weight DMAs issue first on the scalar queue
            nc.scalar.dma_start(ones_1x64[:], ones1[:])
            nc.scalar.dma_start(cosT[:], cos2[:])
            nc.scalar.dma_start(sinT[:], sin2[:])
            nc.scalar.dma_start(tri[:], tri_in[:])
            nc.scalar.dma_start(prope[:], prope2[:])
            nc.scalar.dma_start(id64[:], ident64[:])
            nc.scalar.dma_start(v_nat[:, :, DKV:DKV + 1],
                                colones[:].rearrange("p (s o) -> p s o", o=1))

        # dense contracts 640 rows but only 576 are written by attention;
        # zero the pad rows (their wd rows are 0, but garbage could be NaN)
        nc.vector.memset(attn_sb[64:128, 4, :], 0.0)

        wq_r = wq_bf[:].rearrange("(kt p) r -> p kt r", p=128)
        hs_r = hs_bf[:].rearrange("(kt p) m -> p kt m", p=128)
        wd_r = wd_bf[:].rearrange("(kt p) c -> p kt c", p=128)

        # ---- emission-unit generators ----
        def hs_prefetch(mc):
            """Allocate + start the hs DMA for a chunk ahead of its compute."""
            hsT = hstp.tile([128, KT, MCH], BF16, tag="hsT")
            base = MCH * mc
            for g in range(KT // KTG):
                nc.sync.dma_start(hsT[:, KTG * g:KTG * (g + 1), :],
                                  hs_r[:, KTG * g:KTG * (g + 1), base:base + MCH])
            return hsT

        def gen_qkv(mc, hsT=None, cast_scalar=False):
            """QKV projection + RoPE for one 256-token chunk; yields between
            6-matmul subgroups (~0.64us tensor each). cast_scalar: do the
            PSUM->fusedT cast on the ACT engine (the DVE queue is busy with
            attention epilogues when this runs as filler)."""
            base = MCH * mc
            if hsT is None:
                hsT = hs_prefetch(mc)
            pos0 = base % L

            def rope(rc):
                # RoPE in place on this chunk's q rows (and k rows in rc4)
                x = fusedT[:, rc, base:base + MCH]
                pp = bigp.tile([128, 512], F32, tag="big")
                nc.tensor.matmul(pp[:, :MCH], prope[:], x, start=True, stop=True)
                b32 = workp.tile([128, MCH], F32, tag="b32")
                nc.vector.tensor_mul(b32[:], pp[:, :MCH],
                                     sinT[:, pos0:pos0 + MCH])
                a16 = workp.tile([128, MCH], BF16, tag="a16")
                nc.vector.tensor_mul(a16[:], x, cosT[:, pos0:pos0 + MCH])
                nc.vector.tensor_add(x, a16[:], b32[:])

            pending_rope = None
            for rc in range(RC):
                rp = 128 if rc < 5 else 64
                ps = bigp.tile([128, 512], F32, tag="big")
                for g in range(KT // KTG):
                    for kt in range(KTG * g, KTG * (g + 1)):
                        nc.tensor.matmul(
                            ps[:rp, :MCH], wqT[:, kt, 128 * rc:128 * rc + rp],
                            hsT[:, kt, :], start=(kt == 0), stop=(kt == KT - 1))
                    yield
                    if g == 0 and pending_rope is not None:
                        # emit the previous rc's rope now — its pp matmul needs
                        # that rc's DVE cast, which has had a subgroup to land
                        pending_rope()
                        pending_rope = None
                nc.vector.tensor_copy(fusedT[:rp, rc, base:base + MCH],
                                      ps[:rp, :MCH])
                if rc < 5:
                    pending_rope = (lambda rc=rc: rope(rc))
                    yield
            if pending_rope is not None:
                pending_rope()

        def gen_prep(n):
            """kT_dup + v_nat for batch n (after its QKV+RoPE chunks)."""
            # kT duplicated into both partition halves so lhsT/rhs base
            # partitions match for every head
            nc.scalar.dma_start(kT_dup[0:64, n, :],
                                fusedT[64:128, 4, L * n:L * (n + 1)])
            nc.scalar.dma_start(kT_dup[64:128, n, :],
                                fusedT[64:128, 4, L * n:L * (n + 1)])
            yield
            for jt in range(8):
                tp = bigp.tile([128, 512], BF16, tag="big")
                nc.tensor.transpose(
                    tp[:, 0:64],
                    fusedT[0:64, 5, L * n + 128 * jt:L * n + 128 * (jt + 1)],
                    id64[:])
                nc.vector.tensor_copy(v_nat[:, 8 * n + jt, 0:DKV], tp[:, 0:64])
                yield

        def attn_head(n, h, qc):
            """One attention head, one 512-query block; yields between j-tiles."""
            poff = (64 * h) % 128
            prc = (64 * h) // 128
            kTn = kT_dup[poff:poff + 64, n, :]
            qh = fusedT[poff:poff + 64, prc, L * n:L * (n + 1)]
            av = avp.tile([65, 512], F32, tag="av")
            njt = 4 * (qc + 1)
            pend = None
            for jt in range(njt):
                off = max(0, 128 * jt - 512 * qc)
                sp = spp.tile([128, 512], F32, tag="sp")
                nc.tensor.matmul(
                    sp[:, 0:512 - off],
                    kTn[:, 128 * jt:128 * (jt + 1)],
                    qh[:, 512 * qc + off:512 * (qc + 1)],
                    start=True, stop=True)
                et = expp.tile([128, 512], BF16, tag="exp")
                nc.scalar.activation(
                    et[:, off:512], sp[:, 0:512 - off],
                    mybir.ActivationFunctionType.Exp,
                    scale=1.0 / math.sqrt(DKV))
                if 128 * jt >= 512 * qc:
                    nc.vector.tensor_mul(
                        et[:, off:off + 128], et[:, off:off + 128], tri[:])
                if pend is not None:
                    pjt, po, pet = pend
                    nc.tensor.matmul(
                        av[:, po:512], v_nat[:, 8 * n + pjt, :], pet[:, po:512],
                        start=(pjt == 0), stop=False)
                pend = (jt, off, et)
                yield
            pjt, po, pet = pend
            nc.tensor.matmul(
                av[:, po:512], v_nat[:, 8 * n + pjt, :], pet[:, po:512],
                start=(pjt == 0), stop=True)
            # the custom-DVE reciprocal misreads PSUM inputs — stage the
            # denominator row through SBUF first
            den = recp.tile([1, 512], F32, tag="den", bufs=1)
            nc.vector.tensor_copy(den[:], av[64:65, :])
            rec = recp.tile([1, 512], F32, tag="rec")
            nc.vector.reciprocal_approx_fast(rec[:], den[:])
            if n == 0 and h == 0 and qc == 0:
                nc.sync.dma_start(dbg_rec[:], rec[:])
                nc.sync.dma_start(dbg_den[:], den[:])
            yield
            prb = recp.tile([64, 512], F32, tag="prb")
            nc.gpsimd.partition_broadcast(prb[:], rec[:])
            yield
            nc.vector.tensor_mul(
                attn_sb[poff:poff + 64, prc, L * n + 512 * qc:L * n + 512 * (qc + 1)],
                av[0:64, :], prb[:])
            yield

        def gen_dense(n, mt, pools=None, fine=False):
            """Partial dense for one 128-token tile; yields per column chunk.
            pools: PSUM pools to rotate through (deeper pipeline in the final
            drain). fine: split casts/DMAs in half so the kernel-ending DMA
            receipt lands sooner."""
            tok = L * n + 128 * mt
            pools = pools or [(bigp, "big")]
            col = 0
            for i, w in enumerate(CCH):
                pl, tg = pools[i % len(pools)]
                pa = pl.tile([128, 512], F32, tag=tg)
                for kt in range(QPAD // 128):
                    nc.tensor.matmul(
                        pa[:, :w], attn_sb[:, kt, tok:tok + 128],
                        wdT2[:, kt, col:col + w],
                        start=(kt == 0), stop=(kt == QPAD // 128 - 1))
                ot = otp.tile([128, 512], BF16, tag="ot")
                halves = [(0, w)] if not fine else [(0, w // 2), (w // 2, w)]
                for a, b in halves:
                    if i % 2 == 0:
                        nc.scalar.copy(ot[:, a:b], pa[:, a:b])
                    else:
                        nc.vector.tensor_copy(ot[:, a:b], pa[:, a:b])
                    nc.sync.dma_start(out[tok:tok + 128, col + a:col + b],
                                      ot[:, a:b])
                col += w
                yield

        def run(g):
            for _ in g:
                pass

        def drive(primaries, fillers):
            """Step the primary generators (2 rolling, in order) with one
            filler unit per round to keep the tensor queue saturated.
            Leftover fillers are NOT drained — they carry to the next stage."""
            active = []
            nxt = 0
            while active or nxt < len(primaries):
                while len(active) < 3 and nxt < len(primaries):
                    active.append(primaries[nxt]())
                    nxt += 1
                for g in list(active):
                    try:
                        next(g)
                    except StopIteration:
                        active.remove(g)
                if fillers:
                    try:
                        next(fillers[0])
                    except StopIteration:
                        fillers.pop(0)

        def gen_qkv0_ktmajor():
            """Chunk 0 with kt-major order: all 6 rc tiles accumulate in
            parallel (6 PSUM banks borrowed from the idle attention pools) so
            compute after wq group g needs only wq groups 0..g — the rc-major
            order made rc0 crawl behind the full 6.3 MB wq stream."""
            base = 0
            hsT = hstp.tile([128, KT, MCH], BF16, tag="hsT")
            for a, b in [(0, 1), (1, 3), (3, 6)]:    # tiny first groups
                nc.scalar.dma_start(wqT[:, a:b, :], wq_r[:, a:b, :])
            for a, b in [(0, 3), (3, 6)]:
                nc.sync.dma_start(hsT[:, a:b, :], hs_r[:, a:b, base:base + MCH])
            for g in range(1, KT // KTG):
                h0 = KTG * g
                nc.scalar.dma_start(wqT[:, h0:h0 + KTG // 2, :],
                                    wq_r[:, h0:h0 + KTG // 2, :])
                nc.scalar.dma_start(wqT[:, h0 + KTG // 2:h0 + KTG, :],
                                    wq_r[:, h0 + KTG // 2:h0 + KTG, :])
                nc.sync.dma_start(hsT[:, KTG * g:KTG * (g + 1), :],
                                  hs_r[:, KTG * g:KTG * (g + 1), base:base + MCH])
            load_consts()
            accs = [bigp.tile([128, 512], F32, tag="big", name=f"acc{i}")
                    for i in range(2)]
            accs += [spp.tile([128, 512], F32, tag="sp", name=f"acc{i + 2}")
                     for i in range(3)]
            accs.append(avp.tile([128, 512], F32, tag="av", name="acc5"))
            for g in range(KT // KTG):
                for rc in range(RC):
                    rp = 128 if rc < 5 else 64
                    for kt in range(KTG * g, KTG * (g + 1)):
                        nc.tensor.matmul(
                            accs[rc][:rp, :MCH], wqT[:, kt, 128 * rc:128 * rc + rp],
                            hsT[:, kt, :], start=(kt == 0), stop=(kt == KT - 1))
                    yield
            for rc in range(RC):
                rp = 128 if rc < 5 else 64
                nc.vector.tensor_copy(fusedT[:rp, rc, base:base + MCH],
                                      accs[rc][:rp, :MCH])
            for rc in range(5):
                x = fusedT[:, rc, base:base + MCH]
                pp = bigp.tile([128, 512], F32, tag="big")
                nc.tensor.matmul(pp[:, :MCH], prope[:], x,
                                 start=True, stop=True)
                b32 = workp.tile([128, MCH], F32, tag="b32")
                nc.vector.tensor_mul(b32[:], pp[:, :MCH],
                                     sinT[:, base:base + MCH])
                a16 = workp.tile([128, MCH], BF16, tag="a16")
                nc.vector.tensor_mul(a16[:], x, cosT[:, base:base + MCH])
                nc.vector.tensor_add(x, a16[:], b32[:])
                yield

        # ---- schedule ----
        run(gen_qkv0_ktmajor())
        for mc in range(1, 4):
            run(gen_qkv(mc))
        nc.sync.dma_start(dbg_fus[:], fusedT[:, 0, 0:512])
        for kt in range(QPAD // 128):
            nc.scalar.dma_start(wdT2[:, kt, :], wd_r[:, kt, :])
        run(gen_prep(0))

        # attention b0 (qc0 sweep then qc1 sweep) over QKV b1 fillers
        hsT4 = hs_prefetch(4)
        hsT5 = hs_prefetch(5)
        f1 = [gen_qkv(4, hsT4), gen_qkv(5, hsT5), gen_qkv(6), gen_qkv(7),
              gen_prep(1)]
        drive([(lambda h=h: attn_head(0, h, 0)) for h in range(HPC)], f1)
        drive([(lambda h=h: attn_head(0, h, 1)) for h in range(HPC)], f1)
        for f in f1:
            run(f)

        # attention b1 qc0 over dense-b0 fillers; once the qc0 sweep is done,
        # dense b1 mt0-3 (tokens 0-511) join the filler queue under qc1
        f2 = [gen_dense(0, mt) for mt in range(8)]
        drive([(lambda h=h: attn_head(1, h, 0)) for h in range(HPC)], f2)
        f2 += [gen_dense(1, mt) for mt in range(4)]
        drive([(lambda h=h: attn_head(1, h, 1)) for h in range(HPC)], f2)
        for f in f2:
            run(f)
        drain_pools = [(bigp, "big"), (spp, "sp"), (avp, "av")]
        for mt in range(4, 8):
            run(gen_dense(1, mt, pools=drain_pools, fine=(mt == 7)))

    nc.compile()
    return nc


_NC_CACHE = None


def _get_nc():
    global _NC_CACHE
    if _NC_CACHE is None:
        _NC_CACHE = _build()
    return _NC_CACHE


def _host_inputs(hidden_states, w_qkv, w_dense):
    """Build the per-core input maps (transpose + slice + bf16 cast on host)."""
    hs = np.asarray(hidden_states, dtype=np.float32).reshape(M, D)
    w_qkv = np.asarray(w_qkv, dtype=np.float32)
    w_dense = np.asarray(w_dense, dtype=np.float32)
    hs_bf = np.zeros((DP, M), dtype=ml_dtypes.bfloat16)
    hs_bf[:D, :] = np.ascontiguousarray(hs.T).astype(ml_dtypes.bfloat16)

    # RoPE tables, transposed to [dkv, l], duplicated on partitions 0-63 / 64-127
    inv_freq = 1.0 / (ROPE_BASE ** (np.arange(0, DKV, 2, dtype=np.float32) / DKV))
    t = np.arange(L, dtype=np.float32)
    freqs = np.outer(t, inv_freq)
    emb = np.concatenate([freqs, freqs], axis=-1)        # [L, DKV]
    cosT = np.cos(emb).T.astype(ml_dtypes.bfloat16)      # [DKV, L]
    sinT = np.sin(emb).T.astype(ml_dtypes.bfloat16)
    cos2 = np.concatenate([cosT, cosT], axis=0)          # [128, L]
    sin2 = np.concatenate([sinT, sinT], axis=0)

    # tri[j, q] = 1 if j <= q (within-tile causal mask)
    tri = (np.arange(128)[:, None] <= np.arange(128)[None, :]).astype(
        ml_dtypes.bfloat16)

    # RoPE rotation: (P x)[d] = -x[d+32] (d<32), x[d-32] (d>=32); lhsT = P.T, 2 blocks
    P1 = np.zeros((DKV, DKV), dtype=np.float32)
    for d in range(32):
        P1[d, d + 32] = -1.0
        P1[d + 32, d] = 1.0
    PT = P1.T
    prope2 = np.zeros((128, 128), dtype=np.float32)
    prope2[:64, :64] = PT
    prope2[64:, 64:] = PT

    kv_bf = w_qkv[H * DKV:, :].T.astype(ml_dtypes.bfloat16)   # [D, 128]
    in_maps = []
    for c in range(NCORES):
        h0 = HPC * c
        nh = min(HPC, H - h0)
        wq_loc = np.zeros((DP, RROWS), dtype=ml_dtypes.bfloat16)
        wq_loc[:D, :nh * DKV] = w_qkv[h0 * DKV:(h0 + nh) * DKV, :].T.astype(
            ml_dtypes.bfloat16)
        wq_loc[:D, QROWS:] = kv_bf

        # dense weight rows for this core's heads: w_dense columns
        # [64*h0 : 64*(h0+nh)) transposed, zero-padded to QPAD rows
        wd_loc = np.zeros((QPAD, D), dtype=ml_dtypes.bfloat16)
        wd_loc[:nh * DKV, :] = w_dense[:, DKV * h0:DKV * (h0 + nh)].T.astype(
            ml_dtypes.bfloat16)

        in_maps.append({
            "hs_bf": hs_bf,
            "wq_bf": wq_loc,
            "wd_bf": wd_loc,
            "cos2": cos2,
            "sin2": sin2,
            "tri": tri,
            "prope2": prope2.astype(ml_dtypes.bfloat16),
            "ident64": np.eye(64, dtype=ml_dtypes.bfloat16),
            "colones": np.ones((128, 16), dtype=ml_dtypes.bfloat16),
            "ones1": np.ones((1, 64), dtype=np.float16),
        })
    return in_maps


def kernel(hidden_states, w_qkv, w_dense, _trace=False, _trace_kwargs=None):
    nc = _get_nc()
    in_maps = _host_inputs(hidden_states, w_qkv, w_dense)
    kw = {}
    if _trace:
        kw = dict(trace=True, **(_trace_kwargs or {}))
    res = run_bass_kernel_spmd(nc, in_maps, list(range(NCORES)), **kw)
    full = res.results[0]["out"].astype(np.float32)
    for c in range(1, NCORES):
        full += res.results[c]["out"].astype(np.float32)
    kernel._last_exec_time_ns = res.exec_time_ns
    kernel._last_res = res
    return full.reshape(N, L, D).astype(np.float32)


# revision 43
# speedup vs baseline: 1.0070x; 1.0070x over previous
"""Falcon-style MQA attention (71 heads, 1 KV head, RoPE, causal) on 8 TRN2 NeuronCores.

Sharding: tensor-parallel over query heads (9 per core, core 7 has 8 + a zero-pad
head), the single KV head replicated. Per core: QKV projection for its heads
(+KV), RoPE, causal flash-style attention in transposed layout, then a PARTIAL
dense projection over the core's own head rows for all 4544 output columns.
The host sums the 8 bf16 partial outputs in f32 (contraction-sharded dense =
host reduce); no device collective is needed.

v2 layout: everything bf16 on the PE (fp32r attention matmuls were 3x slower
under HAM throttling), softmax reciprocal via the fast custom-DVE op, dense
partials cast to bf16, all DMA issue on the sync queue, and QKV(b1)/dense
emission finely interleaved into the attention instruction streams so the
tensor engine never idles (keeps the HAM clock gate at 2.4 GHz).

Self-contained: hardcodes all shapes; needs only numpy + ml_dtypes + concourse.
"""

import math
from contextlib import ExitStack

import numpy as np
import ml_dtypes

import concourse.bass as bass
import concourse.mybir as mybir
import concourse.tile as tile
from concourse import bacc
from concourse.bass_utils import run_bass_kernel_spmd

NCORES = 8
N, L, D = 2, 1024, 4544
H, DKV = 71, 64
M = N * L                    # 2048 tokens
DP = 4608                    # D padded to 36*128
KT = DP // 128               # 36 contraction tiles for QKV
HPC = 9                      # head slots per core (core 7: 8 real + 1 zero-pad)
QROWS = HPC * DKV            # 576 attention rows per core
QPAD = 640                   # padded to 5*128 for the dense contraction
RROWS = QROWS + 2 * DKV      # 704 fused rows per core (q + k + v)
RC = 6                       # row-chunks of fusedT (5x128 + 64)
MCH = 256                    # QKV token-chunk width
KTG = 6                      # kt-subgroup size (DMA + matmul granularity)
ROPE_BASE = 10000.0

F32 = mybir.dt.float32
F16 = mybir.dt.float16
BF16 = mybir.dt.bfloat16

CCH = [512] * 8 + [448]      # dense column chunks (sum = 4544)


def _build():
    nc = bacc.Bacc("TRN2", target_bir_lowering=False, debug=False, num_devices=NCORES)

    hs_bf = nc.dram_tensor("hs_bf", [DP, M], BF16, kind="ExternalInput")      # hs.T
    wq_bf = nc.dram_tensor("wq_bf", [DP, RROWS], BF16, kind="ExternalInput")  # wq_loc.T
    wd_bf = nc.dram_tensor("wd_bf", [QPAD, D], BF16, kind="ExternalInput")    # wd rows for local heads
    cos2 = nc.dram_tensor("cos2", [128, L], BF16, kind="ExternalInput")
    sin2 = nc.dram_tensor("sin2", [128, L], BF16, kind="ExternalInput")
    tri_in = nc.dram_tensor("tri", [128, 128], BF16, kind="ExternalInput")
    prope2 = nc.dram_tensor("prope2", [128, 128], BF16, kind="ExternalInput")
    ident64 = nc.dram_tensor("ident64", [64, 64], BF16, kind="ExternalInput")
    colones = nc.dram_tensor("colones", [128, 16], BF16, kind="ExternalInput")
    ones1 = nc.dram_tensor("ones1", [1, 64], F16, kind="ExternalInput")
    out = nc.dram_tensor("out", [M, D], BF16, kind="ExternalOutput")
    dbg_rec = nc.dram_tensor("dbg_rec", [1, 512], F32, kind="ExternalOutput")
    dbg_den = nc.dram_tensor("dbg_den", [1, 512], F32, kind="ExternalOutput")
    dbg_fus = nc.dram_tensor("dbg_fus", [128, 512], BF16, kind="ExternalOutput")

    with tile.TileContext(nc) as tc, ExitStack() as top:
        constp = top.enter_context(tc.tile_pool(name="const", bufs=1))
        workp = top.enter_context(tc.tile_pool(name="work", bufs=2))
        hstp = top.enter_context(tc.tile_pool(name="hst", bufs=2))
        otp = top.enter_context(tc.tile_pool(name="ot", bufs=4))
        recp = top.enter_context(tc.tile_pool(name="rec", bufs=2))
        expp = top.enter_context(tc.tile_pool(name="exps", bufs=5))
        bigp = top.enter_context(tc.tile_pool(name="big", bufs=2, space="PSUM"))
        spp = top.enter_context(tc.tile_pool(name="spp", bufs=3, space="PSUM"))
        avp = top.enter_context(tc.tile_pool(name="avp", bufs=3, space="PSUM"))

        # ---- persistent tiles ----
        fusedp = top.enter_context(tc.tile_pool(name="fused", bufs=1))
        fusedT = fusedp.tile([128, RC, M], BF16)
        attnp = top.enter_context(tc.tile_pool(name="attn", bufs=1))
        attn_sb = attnp.tile([128, QPAD // 128, M], BF16)
        wqp = top.enter_context(tc.tile_pool(name="wq", bufs=1))
        wqT = wqp.tile([128, KT, RROWS], BF16)
        wdp = top.enter_context(tc.tile_pool(name="wd", bufs=1))
        wdT2 = wdp.tile([128, QPAD // 128, D], BF16)
        kvp = top.enter_context(tc.tile_pool(name="kv", bufs=1))
        kT_dup = kvp.tile([128, N, L], BF16)
        v_nat = kvp.tile([128, N * 8, DKV + 1], BF16)

        # ---- constants ----
        cosT = constp.tile([128, L], BF16)
        sinT = constp.tile([128, L], BF16)
        tri = constp.tile([128, 128], BF16)
        prope = constp.tile([128, 128], BF16)
        id64 = constp.tile([64, 64], BF16)
        ones_1x64 = constp.tile([1, 64], F16)
        def load_consts():
            # emitted after the first wq groups so the startup-critical
            # weight DMAs issue first on the scalar queue
            nc.scalar.dma_start(ones_1x64[:], ones1[:])
            nc.scalar.dma_start(cosT[:], cos2[:])
            nc.scalar.dma_start(sinT[:], sin2[:])
            nc.scalar.dma_start(tri[:], tri_in[:])
            nc.scalar.dma_start(prope[:], prope2[:])
            nc.scalar.dma_start(id64[:], ident64[:])
            nc.scalar.dma_start(v_nat[:, :, DKV:DKV + 1],
                                colones[:].rearrange("p (s o) -> p s o", o=1))

        # dense contracts 640 rows but only 576 are written by attention;
        # zero the pad rows (their wd rows are 0, but garbage could be NaN)
        nc.vector.memset(attn_sb[64:128, 4, :], 0.0)

        wq_r = wq_bf[:].rearrange("(kt p) r -> p kt r", p=128)
        hs_r = hs_bf[:].rearrange("(kt p) m -> p kt m", p=128)
        wd_r = wd_bf[:].rearrange("(kt p) c -> p kt c", p=128)

        # ---- emission-unit generators ----
        def hs_prefetch(mc):
            """Allocate + start the hs DMA for a chunk ahead of its compute."""
            hsT = hstp.tile([128, KT, MCH], BF16, tag="hsT")
            base = MCH * mc
            for g in range(KT // KTG):
                nc.sync.dma_start(hsT[:, KTG * g:KTG * (g + 1), :],
                                  hs_r[:, KTG * g:KTG * (g + 1), base:base + MCH])
            return hsT

        def gen_qkv(mc, hsT=None, cast_scalar=False):
            """QKV projection + RoPE for one 256-token chunk; yields between
            6-matmul subgroups (~0.64us tensor each). cast_scalar: do the
            PSUM->fusedT cast on the ACT engine (the DVE queue is busy with
            attention epilogues when this runs as filler)."""
            base = MCH * mc
            if hsT is None:
                hsT = hs_prefetch(mc)
            pos0 = base % L

            def rope(rc):
                # RoPE in place on this chunk's q rows (and k rows in rc4)
                x = fusedT[:, rc, base:base + MCH]
                pp = bigp.tile([128, 512], F32, tag="big")
                nc.tensor.matmul(pp[:, :MCH], prope[:], x, start=True, stop=True)
                b32 = workp.tile([128, MCH], F32, tag="b32")
                nc.vector.tensor_mul(b32[:], pp[:, :MCH],
                                     sinT[:, pos0:pos0 + MCH])
                a16 = workp.tile([128, MCH], BF16, tag="a16")
                nc.vector.tensor_mul(a16[:], x, cosT[:, pos0:pos0 + MCH])
                nc.vector.tensor_add(x, a16[:], b32[:])

            pending_rope = None
            for rc in range(RC):
                rp = 128 if rc < 5 else 64
                ps = bigp.tile([128, 512], F32, tag="big")
                for g in range(KT // KTG):
                    for kt in range(KTG * g, KTG * (g + 1)):
                        nc.tensor.matmul(
                            ps[:rp, :MCH], wqT[:, kt, 128 * rc:128 * rc + rp],
                            hsT[:, kt, :], start=(kt == 0), stop=(kt == KT - 1))
                    yield
                    if g == 0 and pending_rope is not None:
                        # emit the previous rc's rope now — its pp matmul needs
                        # that rc's DVE cast, which has had a subgroup to land
                        pending_rope()
                        pending_rope = None
                nc.vector.tensor_copy(fusedT[:rp, rc, base:base + MCH],
                                      ps[:rp, :MCH])
                if rc < 5:
                    pending_rope = (lambda rc=rc: rope(rc))
                    yield
            if pending_rope is not None:
                pending_rope()

        def gen_prep(n):
            """kT_dup + v_nat for batch n (after its QKV+RoPE chunks)."""
            # kT duplicated into both partition halves so lhsT/rhs base
            # partitions match for every head
            nc.scalar.dma_start(kT_dup[0:64, n, :],
                                fusedT[64:128, 4, L * n:L * (n + 1)])
            nc.scalar.dma_start(kT_dup[64:128, n, :],
                                fusedT[64:128, 4, L * n:L * (n + 1)])
            yield
            for jt in range(8):
                tp = bigp.tile([128, 512], BF16, tag="big")
                nc.tensor.transpose(
                    tp[:, 0:64],
                    fusedT[0:64, 5, L * n + 128 * jt:L * n + 128 * (jt + 1)],
                    id64[:])
                nc.vector.tensor_copy(v_nat[:, 8 * n + jt, 0:DKV], tp[:, 0:64])
                yield

        def attn_head(n, h, qc):
            """One attention head, one 512-query block; yields between j-tiles."""
            poff = (64 * h) % 128
            prc = (64 * h) // 128
            kTn = kT_dup[poff:poff + 64, n, :]
            qh = fusedT[poff:poff + 64, prc, L * n:L * (n + 1)]
            av = avp.tile([65, 512], F32, tag="av")
            njt = 4 * (qc + 1)
            pend = None
            for jt in range(njt):
                off = max(0, 128 * jt - 512 * qc)
                sp = spp.tile([128, 512], F32, tag="sp")
                nc.tensor.matmul(
                    sp[:, 0:512 - off],
                    kTn[:, 128 * jt:128 * (jt + 1)],
                    qh[:, 512 * qc + off:512 * (qc + 1)],
                    start=True, stop=True)
                et = expp.tile([128, 512], BF16, tag="exp")
                nc.scalar.activation(
                    et[:, off:512], sp[:, 0:512 - off],
                    mybir.ActivationFunctionType.Exp,
                    scale=1.0 / math.sqrt(DKV))
                if 128 * jt >= 512 * qc:
                    nc.vector.tensor_mul(
                        et[:, off:off + 128], et[:, off:off + 128], tri[:])
                if pend is not None:
                    pjt, po, pet = pend
                    nc.tensor.matmul(
                        av[:, po:512], v_nat[:, 8 * n + pjt, :], pet[:, po:512],
                        start=(pjt == 0), stop=False)
                pend = (jt, off, et)
                yield
            pjt, po, pet = pend
            nc.tensor.matmul(
                av[:, po:512], v_nat[:, 8 * n + pjt, :], pet[:, po:512],
                start=(pjt == 0), stop=True)
            # the custom-DVE reciprocal misreads PSUM inputs — stage the
            # denominator row through SBUF first
            den = recp.tile([1, 512], F32, tag="den", bufs=1)
            nc.vector.tensor_copy(den[:], av[64:65, :])
            rec = recp.tile([1, 512], F32, tag="rec")
            nc.vector.reciprocal_approx_fast(rec[:], den[:])
            if n == 0 and h == 0 and qc == 0:
                nc.sync.dma_start(dbg_rec[:], rec[:])
                nc.sync.dma_start(dbg_den[:], den[:])
            yield
            prb = recp.tile([64, 512], F32, tag="prb")
            nc.gpsimd.partition_broadcast(prb[:], rec[:])
            yield
            nc.vector.tensor_mul(
                attn_sb[poff:poff + 64, prc, L * n + 512 * qc:L * n + 512 * (qc + 1)],
                av[0:64, :], prb[:])
            yield

        def gen_dense(n, mt, pools=None):
            """Partial dense for one 128-token tile; yields per column chunk.
            pools: PSUM pools to rotate through (deeper pipeline in the final
            drain when attention no longer needs sp/av/pr banks)."""
            tok = L * n + 128 * mt
            pools = pools or [(bigp, "big")]
            col = 0
            for i, w in enumerate(CCH):
                pl, tg = pools[i % len(pools)]
                pa = pl.tile([128, 512], F32, tag=tg)
                for kt in range(QPAD // 128):
                    nc.tensor.matmul(
                        pa[:, :w], attn_sb[:, kt, tok:tok + 128],
                        wdT2[:, kt, col:col + w],
                        start=(kt == 0), stop=(kt == QPAD // 128 - 1))
                ot = otp.tile([128, 512], BF16, tag="ot")
                if i % 2 == 0:
                    nc.scalar.copy(ot[:, :w], pa[:, :w])
                else:
                    nc.vector.tensor_copy(ot[:, :w], pa[:, :w])
                nc.sync.dma_start(out[tok:tok + 128, col:col + w], ot[:, :w])
                col += w
                yield

        def run(g):
            for _ in g:
                pass

        def drive(primaries, fillers):
            """Step the primary generators (2 rolling, in order) with one
            filler unit per round to keep the tensor queue saturated.
            Leftover fillers are NOT drained — they carry to the next stage."""
            active = []
            nxt = 0
            while active or nxt < len(primaries):
                while len(active) < 3 and nxt < len(primaries):
                    active.append(primaries[nxt]())
                    nxt += 1
                for g in list(active):
                    try:
                        next(g)
                    except StopIteration:
                        active.remove(g)
                if fillers:
                    try:
                        next(fillers[0])
                    except StopIteration:
                        fillers.pop(0)

        def gen_qkv0_ktmajor():
            """Chunk 0 with kt-major order: all 6 rc tiles accumulate in
            parallel (6 PSUM banks borrowed from the idle attention pools) so
            compute after wq group g needs only wq groups 0..g — the rc-major
            order made rc0 crawl behind the full 6.3 MB wq stream."""
            base = 0
            hsT = hstp.tile([128, KT, MCH], BF16, tag="hsT")
            for a, b in [(0, 1), (1, 3), (3, 6)]:    # tiny first groups
                nc.scalar.dma_start(wqT[:, a:b, :], wq_r[:, a:b, :])
            for a, b in [(0, 3), (3, 6)]:
                nc.sync.dma_start(hsT[:, a:b, :], hs_r[:, a:b, base:base + MCH])
            for g in range(1, KT // KTG):
                h0 = KTG * g
                nc.scalar.dma_start(wqT[:, h0:h0 + KTG // 2, :],
                                    wq_r[:, h0:h0 + KTG // 2, :])
                nc.scalar.dma_start(wqT[:, h0 + KTG // 2:h0 + KTG, :],
                                    wq_r[:, h0 + KTG // 2:h0 + KTG, :])
                nc.sync.dma_start(hsT[:, KTG * g:KTG * (g + 1), :],
                                  hs_r[:, KTG * g:KTG * (g + 1), base:base + MCH])
            load_consts()
            accs = [bigp.tile([128, 512], F32, tag="big", name=f"acc{i}")
                    for i in range(2)]
            accs += [spp.tile([128, 512], F32, tag="sp", name=f"acc{i + 2}")
                     for i in range(3)]
            accs.append(avp.tile([128, 512], F32, tag="av", name="acc5"))
            for g in range(KT // KTG):
                for rc in range(RC):
                    rp = 128 if rc < 5 else 64
                    for kt in range(KTG * g, KTG * (g + 1)):
                        nc.tensor.matmul(
                            accs[rc][:rp, :MCH], wqT[:, kt, 128 * rc:128 * rc + rp],
                            hsT[:, kt, :], start=(kt == 0), stop=(kt == KT - 1))
                    yield
            for rc in range(RC):
                rp = 128 if rc < 5 else 64
                nc.vector.tensor_copy(fusedT[:rp, rc, base:base + MCH],
                                      accs[rc][:rp, :MCH])
            for rc in range(5):
                x = fusedT[:, rc, base:base + MCH]
                pp = bigp.tile([128, 512], F32, tag="big")
                nc.tensor.matmul(pp[:, :MCH], prope[:], x,
                                 start=True, stop=True)
                b32 = workp.tile([128, MCH], F32, tag="b32")
                nc.vector.tensor_mul(b32[:], pp[:, :MCH],
                                     sinT[:, base:base + MCH])
                a16 = workp.tile([128, MCH], BF16, tag="a16")
                nc.vector.tensor_mul(a16[:], x, cosT[:, base:base + MCH])
                nc.vector.tensor_add(x, a16[:], b32[:])
                yield

        # ---- schedule ----
        run(gen_qkv0_ktmajor())
        for mc in range(1, 4):
            run(gen_qkv(mc))
        nc.sync.dma_start(dbg_fus[:], fusedT[:, 0, 0:512])
        for kt in range(QPAD // 128):
            nc.scalar.dma_start(wdT2[:, kt, :], wd_r[:, kt, :])
        run(gen_prep(0))

        # attention b0 (qc0 sweep then qc1 sweep) over QKV b1 fillers
        hsT4 = hs_prefetch(4)
        hsT5 = hs_prefetch(5)
        f1 = [gen_qkv(4, hsT4), gen_qkv(5, hsT5), gen_qkv(6), gen_qkv(7),
              gen_prep(1)]
        drive([(lambda h=h: attn_head(0, h, 0)) for h in range(HPC)], f1)
        drive([(lambda h=h: attn_head(0, h, 1)) for h in range(HPC)], f1)
        for f in f1:
            run(f)

        # attention b1 qc0 over dense-b0 fillers; once the qc0 sweep is done,
        # dense b1 mt0-3 (tokens 0-511) join the filler queue under qc1
        f2 = [gen_dense(0, mt) for mt in range(8)]
        drive([(lambda h=h: attn_head(1, h, 0)) for h in range(HPC)], f2)
        f2 += [gen_dense(1, mt) for mt in range(4)]
        drive([(lambda h=h: attn_head(1, h, 1)) for h in range(HPC)], f2)
        for f in f2:
            run(f)
        drain_pools = [(bigp, "big"), (spp, "sp"), (avp, "av")]
        for mt in range(4, 8):
            run(gen_dense(1, mt, pools=drain_pools))

    nc.compile()
    return nc


_NC_CACHE = None


def _get_nc():
    global _NC_CACHE
    if _NC_CACHE is None:
        _NC_CACHE = _build()
    return _NC_CACHE


def _host_inputs(hidden_states, w_qkv, w_dense):
    """Build the per-core input maps (transpose + slice + bf16 cast on host)."""
    hs = np.asarray(hidden_states, dtype=np.float32).reshape(M, D)
    w_qkv = np.asarray(w_qkv, dtype=np.float32)
    w_dense = np.asarray(w_dense, dtype=np.float32)
    hs_bf = np.zeros((DP, M), dtype=ml_dtypes.bfloat16)
    hs_bf[:D, :] = np.ascontiguousarray(hs.T).astype(ml_dtypes.bfloat16)

    # RoPE tables, transposed to [dkv, l], duplicated on partitions 0-63 / 64-127
    inv_freq = 1.0 / (ROPE_BASE ** (np.arange(0, DKV, 2, dtype=np.float32) / DKV))
    t = np.arange(L, dtype=np.float32)
    freqs = np.outer(t, inv_freq)
    emb = np.concatenate([freqs, freqs], axis=-1)        # [L, DKV]
    cosT = np.cos(emb).T.astype(ml_dtypes.bfloat16)      # [DKV, L]
    sinT = np.sin(emb).T.astype(ml_dtypes.bfloat16)
    cos2 = np.concatenate([cosT, cosT], axis=0)          # [128, L]
    sin2 = np.concatenate([sinT, sinT], axis=0)

    # tri[j, q] = 1 if j <= q (within-tile causal mask)
    tri = (np.arange(128)[:, None] <= np.arange(128)[None, :]).astype(
        ml_dtypes.bfloat16)

    # RoPE rotation: (P x)[d] = -x[d+32] (d<32), x[d-32] (d>=32); lhsT = P.T, 2 blocks
    P1 = np.zeros((DKV, DKV), dtype=np.float32)
    for d in range(32):
        P1[d, d + 32] = -1.0
        P1[d + 32, d] = 1.0
    PT = P1.T
    prope2 = np.zeros((128, 128), dtype=np.float32)
    prope2[:64, :64] = PT
    prope2[64:, 64:] = PT

    kv_bf = w_qkv[H * DKV:, :].T.astype(ml_dtypes.bfloat16)   # [D, 128]
    in_maps = []
    for c in range(NCORES):
        h0 = HPC * c
        nh = min(HPC, H - h0)
        wq_loc = np.zeros((DP, RROWS), dtype=ml_dtypes.bfloat16)
        wq_loc[:D, :nh * DKV] = w_qkv[h0 * DKV:(h0 + nh) * DKV, :].T.astype(
            ml_dtypes.bfloat16)
        wq_loc[:D, QROWS:] = kv_bf

        # dense weight rows for this core's heads: w_dense columns
        # [64*h0 : 64*(h0+nh)) transposed, zero-padded to QPAD rows
        wd_loc = np.zeros((QPAD, D), dtype=ml_dtypes.bfloat16)
        wd_loc[:nh * DKV, :] = w_dense[:, DKV * h0:DKV * (h0 + nh)].T.astype(
            ml_dtypes.bfloat16)

        in_maps.append({
            "hs_bf": hs_bf,
            "wq_bf": wq_loc,
            "wd_bf": wd_loc,
            "cos2": cos2,
            "sin2": sin2,
            "tri": tri,
            "prope2": prope2.astype(ml_dtypes.bfloat16),
            "ident64": np.eye(64, dtype=ml_dtypes.bfloat16),
            "colones": np.ones((128, 16), dtype=ml_dtypes.bfloat16),
            "ones1": np.ones((1, 64), dtype=np.float16),
        })
    return in_maps


def kernel(hidden_states, w_qkv, w_dense, _trace=False, _trace_kwargs=None):
    nc = _get_nc()
    in_maps = _host_inputs(hidden_states, w_qkv, w_dense)
    kw = {}
    if _trace:
        kw = dict(trace=True, **(_trace_kwargs or {}))
    res = run_bass_kernel_spmd(nc, in_maps, list(range(NCORES)), **kw)
    full = res.results[0]["out"].astype(np.float32)
    for c in range(1, NCORES):
        full += res.results[c]["out"].astype(np.float32)
    kernel._last_exec_time_ns = res.exec_time_ns
    kernel._last_res = res
    return full.reshape(N, L, D).astype(np.float32)
